# revision 1
# baseline (speedup 1.0000x reference)
"""Trainium2 Bass kernel for multi-head quadratic spatial attention.

Problem: q,k,v [b=8, heads=8, h=32, w=32, d=64] fp32; full attention over
the 1024-position spatial grid independently per (b, head); output
[b, h, w, heads*d].

Sharding: data-parallel over batch — core c handles b=c (8 heads of
[1024, 64] attention per core), no cross-core communication.

Per-core pipeline (heads processed in PAIRS; matmuls bf16 with fp32 PSUM
accumulation). The PE executes serially on this toolchain, so the design
minimizes streamed columns + instruction count and keeps the HAM clock
gate warm (no transpose-heavy stretches > ~3.4us, dummy-matmul warm-up):
  - p-major seq tiling (seq = p*8 + t); ONE 4D casting DMA per (tensor,
    pair) interleaving the two heads -> 3 gpsimd triggers per pair
  - 40 dummy ident matmuls warm the PE clock gate (1.2 -> 2.4 GHz) while
    the first DMAs land
  - pair-interleaved natural tiles [128, t, 2, d]: one [128,128] PE
    transpose per block yields head A's d-rows on partitions 0:64 and
    B's on 64:128 — the packed pair layout mm1 wants
  - mm1 row-tiled: head A contracts on PE rows 0:64, head B on 64:128
    -> St [128, 1024] fp32 (separate tiles, freed by their own exp)
  - exp on ScalarE (activation Exp); optional per-(jb, head) offload to
    VectorE via the Schraudolph bit-trick (fused tensor_scalar
    mult+add -> int16 == bf16 exp approx) to unload the ScalarE
  - mm2: lhsT = [V | 1] j-chunk [128, 65] bf16, rhs = Pt slices ->
    accumulate PSUM Ot [65, 512] per i-half; row 64 = softmax sums
  - epilogue in bf16: ot copy on VectorE, PE transposes back (FWL), one
    batched reciprocal [128,4] + per-block tensor_scalar normalize into
    fp32 ostage, stores on the sync HWDGE ring
"""

from contextlib import ExitStack

import numpy as np

F32 = None
BF16 = None
I16 = None

_cache = {}

# Schraudolph exp in bf16 bit-space: bf16_bits(exp(s*x)) ~= round(x*A + B)
# A = s * 2^7/ln2, B = 2^7*(127 - sigma), sigma = 0.0430 balances the
# piecewise-linear 2^frac error (max rel err ~3%, mostly cancelling in the
# softmax average).
SCALE = 64.0 ** -0.5
SCHRAUD_A = SCALE * 128.0 / float(np.log(2.0))
SCHRAUD_B = 128.0 * (127.0 - 0.0430)

N_WARM = 20  # dummy matmuls to flip the PE HAM clock gate before real work

# (jb, head-in-pair) St tiles exp'd on VectorE via the Schraudolph bit-trick
# instead of ScalarE's exact exp. Each entry moves 1/16 of the attention
# weight mass to a ~3% weight-error approximation (mostly cancelling in the
# softmax average) and takes ~1.1us/pair off the ScalarE critical path.
SCHRAUD_TILES = frozenset()


def _imports():
    global F32, BF16, I16
    import concourse.bass as bass
    import concourse.tile as tile
    from concourse import mybir
    from concourse.masks import make_identity

    F32 = mybir.dt.float32
    BF16 = mybir.dt.bfloat16
    I16 = mybir.dt.int16
    return bass, tile, mybir, make_identity


def _split_multi_waits(nc, mybir):
    """Walrus in this container supports only ONE sync-wait per instruction.
    Hoist extra waits onto same-engine InstNoOp's inserted just before."""
    ctr = 0
    for f in nc.m.functions:
        for bb in f.blocks:
            insts = bb.instructions
            if not any(
                i.sync_info and i.sync_info.on_wait and len(i.sync_info.on_wait) > 1
                for i in insts
            ):
                continue
            out = []
            for inst in insts:
                si = inst.sync_info
                waits = list(si.on_wait) if si and si.on_wait else []
                if len(waits) > 1:
                    for w in waits[:-1]:
                        ctr += 1
                        nop = mybir.InstNoOp(
                            name=f"I-wsplit-{ctr}",
                            engine=inst.engine,
                            ins=[],
                            outs=[],
                            sync_info=mybir.SyncInfo(on_wait=[w], on_update=[]),
                        )
                        nc.register_instruction(nop)
                        out.append(nop)
                    si.on_wait = waits[-1:]
                out.append(inst)
            bb.instructions = out


def _build_nc(heads=8, seq=1024, d=64):
    bass, tile, mybir, make_identity = _imports()
    assert heads % 2 == 0 and seq == 1024 and d == 64
    nt = seq // 128          # 8 blocks of 128 positions
    nh = seq // 512          # 2 i-halves of 512
    dv = d + 1
    TS_MULT = mybir.AluOpType.mult
    TS_ADD = mybir.AluOpType.add

    nc = bass.Bass(trn_type="TRN2", target_bir_lowering=False)
    q_d = nc.dram_tensor("q", [heads, seq, d], F32, kind="ExternalInput")
    k_d = nc.dram_tensor("k", [heads, seq, d], F32, kind="ExternalInput")
    v_d = nc.dram_tensor("v", [heads, seq, d], F32, kind="ExternalInput")
    o_d = nc.dram_tensor("out", [seq, heads * d], F32, kind="ExternalOutput")

    # p-major: seq = p*nt + t; per-(p, t) HBM runs are 256B contiguous
    q_ap = q_d[:].rearrange("n (p t) d -> n p t d", p=128)
    k_ap = k_d[:].rearrange("n (p t) d -> n p t d", p=128)
    v_ap = v_d[:].rearrange("n (p t) d -> n p t d", p=128)
    o_ap = o_d[:].rearrange("(p t) c -> p t c", p=128)

    with tile.TileContext(nc) as tc, ExitStack() as ctx:
        consts = ctx.enter_context(tc.tile_pool(name="consts", bufs=1))
        nat = ctx.enter_context(tc.tile_pool(name="nat", bufs=2))
        dmaj = ctx.enter_context(tc.tile_pool(name="dmaj", bufs=2))
        ptp = ctx.enter_context(tc.tile_pool(name="ptp", bufs=36))
        otp = ctx.enter_context(tc.tile_pool(name="otp", bufs=3))
        outp = ctx.enter_context(tc.tile_pool(name="outp", bufs=3))
        small = ctx.enter_context(tc.tile_pool(name="small", bufs=4))

        # PSUM banks: st 2x2 (0-3) + oacc/ob/warm 2x1 (4-5) + tp 2x1 (6-7)
        st_ps = ctx.enter_context(tc.tile_pool(name="st_ps", bufs=2, space="PSUM"))
        oa_ps = ctx.enter_context(tc.tile_pool(name="oa_ps", bufs=2, space="PSUM"))
        tp_ps = ctx.enter_context(tc.tile_pool(name="tp_ps", bufs=2, space="PSUM"))

        ident_bf = consts.tile([128, 128], BF16)
        make_identity(nc, ident_bf[:])
        ident_f32 = consts.tile([128, 128], F32)
        make_identity(nc, ident_f32[:])

        # Warm-up / filler matmuls keep the PE HAM clock gate at 2.4 GHz:
        # an idle (or transpose-only) stretch > ~3.4us re-throttles the PE
        # clock to 1.2 GHz for the next several microseconds. wsrc is
        # memset-ready within ~200ns of kernel start. N=512 streams give
        # ~213ns of HAM-counted busy per filler instruction.
        wsrc = consts.tile([128, 512], BF16)
        nc.vector.memset(wsrc[:], 0.25)

        def pe_filler(n):
            # fresh tile per burst: fillers WAR-chain only onto transient
            # transpose tiles, never onto live oacc accumulators
            t = tp_ps.tile([128, 512], F32, tag="tp", name="warm")
            for _ in range(n):
                nc.tensor.matmul(
                    t[:], wsrc[:, 0:128], wsrc[:], start=True, stop=True
                )

        pe_filler(N_WARM)

        def load_and_transpose(pair):
            """DMA pair inputs (bf16 cast, one 4D DMA per tensor) and build
            packed d-major tiles: head A on partitions 0:64, head B on
            64:128 (one [128,128] PE transpose per block)."""
            st8 = {"heads": (2 * pair, 2 * pair + 1), "v": None, "pts": [],
                   "oacc": {}, "ostage": {}}
            # pair-interleaved natural tiles: [..., 2, d] with head A at
            # index 0 and head B at 1, so one [128, 128] PE transpose of a
            # block yields A's d-rows on partitions 0:64 and B's on 64:128.
            qp = nat.tile([128, nt, 2, d], BF16, tag="qp")
            kp = nat.tile([128, nt, 2, d], BF16, tag="kp")
            hh = nt // 2
            if pair == 0:
                # halved loads, first halves of BOTH heads first, so the
                # first transpose group can start after ~2 trigger slots
                for lo, hi in ((0, hh), (hh, nt)):
                    for src_ap, dst in ((q_ap, qp), (k_ap, kp)):
                        for idx, n in enumerate(st8["heads"]):
                            nc.gpsimd.dma_start(
                                out=dst[:, lo:hi, idx, :], in_=src_ap[n, :, lo:hi]
                            )
            else:
                for idx, n in enumerate(st8["heads"]):
                    nc.gpsimd.dma_start(out=qp[:, :, idx, :], in_=q_ap[n])
                    nc.gpsimd.dma_start(out=kp[:, :, idx, :], in_=k_ap[n])
            vp = nat.tile([128, nt, 2, dv], BF16, tag="vp")
            # ones columns for the softmax-denominator trick
            nc.vector.memset(vp[:, :, :, d : d + 1], 1.0)
            for idx, n in enumerate(st8["heads"]):
                nc.gpsimd.dma_start(out=vp[:, :, idx, 0:d], in_=v_ap[n])
            st8["v"] = vp
            qt = dmaj.tile([128, seq], BF16, tag="qt")
            kt = dmaj.tile([128, seq], BF16, tag="kt")
            for g in range(nt // 4):
                for src, dst in ((qp, qt), (kp, kt)):
                    tp = tp_ps.tile([128, 512], BF16, tag="tp")
                    for u in range(4):
                        t = g * 4 + u
                        nc.tensor.transpose(
                            tp[:, u * 128 : (u + 1) * 128],
                            src[:, t, :, :],
                            ident_bf[:],
                        )
                    nc.vector.tensor_copy(
                        out=dst[:, g * 512 : (g + 1) * 512], in_=tp[:]
                    )
                    if pair == 0:
                        # PE is otherwise DMA-bound here; keep the clock warm
                        pe_filler(1)
            st8["qt"], st8["kt"] = qt, kt
            return st8

        def mm1_exp(s, jb):
            """Row-tiled pair mm1 into per-head St tiles + per-head exp.
            Separate St tiles mean head A's tile is released as soon as its
            own exp finishes."""
            qt, kt = s["qt"], s["kt"]
            stA = st_ps.tile([128, seq], F32, name="stA", tag="st")
            stB = st_ps.tile([128, seq], F32, name="stB", tag="st")
            for c in range(nh):
                nc.tensor.matmul(
                    stA[:, c * 512 : (c + 1) * 512],
                    kt[0:64, jb * 128 : (jb + 1) * 128],
                    qt[0:64, c * 512 : (c + 1) * 512],
                    start=True,
                    stop=True,
                )
                nc.tensor.matmul(
                    stB[:, c * 512 : (c + 1) * 512],
                    kt[64:128, jb * 128 : (jb + 1) * 128],
                    qt[64:128, c * 512 : (c + 1) * 512],
                    start=True,
                    stop=True,
                )
            for idx, st in enumerate((stA, stB)):
                pt = ptp.tile([128, seq], BF16, name="pt", tag="pt")
                if (jb, idx) in SCHRAUD_TILES:
                    nc.vector.tensor_scalar(
                        out=pt[:].bitcast(I16),
                        in0=st[:],
                        scalar1=SCHRAUD_A,
                        scalar2=SCHRAUD_B,
                        op0=TS_MULT,
                        op1=TS_ADD,
                    )
                else:
                    nc.scalar.activation(
                        out=pt[:],
                        in_=st[:],
                        func=mybir.ActivationFunctionType.Exp,
                        scale=SCALE,
                    )
                s["pts"].append((jb, idx, pt))

        # slot s -> (group, phase); phase-1 slots sit at s>=2 so they only
        # run once all 8 jbs' Pt tiles exist (6-jb-shifted pipeline), while
        # at most 2 groups' oacc accumulators are ever live.
        SLOT_ORDER = [(0, 0), (1, 0), (0, 1), (1, 1), (2, 0), (3, 0), (2, 1), (3, 1)]

        def mm2_slot(s, slot, tail=False):
            """One PE-stream slot of the pair's mm2: 4 accumulating
            matmuls of a (head, half) group; epilogue on the closing
            phase."""
            g, phase = SLOT_ORDER[slot]
            idx, half = g // 2, g % 2
            if phase == 0:
                s["oacc"][g] = oa_ps.tile([dv, 512], F32, name="oacc", tag="oacc")
            oacc = s["oacc"][g]
            off = half * 512
            for jj in range(4):
                jb = phase * 4 + jj
                jb2, idx2, pt = s["pts"][jb * 2 + idx]
                assert jb2 == jb and idx2 == idx
                nc.tensor.matmul(
                    oacc[:],
                    s["v"][:, jb, idx, :],
                    pt[:, off : off + 512],
                    start=(jb == 0),
                    stop=(jb == nt - 1),
                )
            if phase == 1:
                _epilogue(s, idx, half, oacc, tail=tail)

        def _epilogue(s, idx, half, oacc, tail=False):
            n = s["heads"][idx]
            if idx not in s["ostage"]:
                s["ostage"][idx] = outp.tile(
                    [128, nt, d], F32, name="ostage", tag="ostage"
                )
            ostage = s["ostage"][idx]
            ot = otp.tile([dv, 512], BF16, tag="ot")
            if tail:
                # ScalarE is idle after its last exp; shorten the tail chain
                nc.scalar.copy(out=ot[:], in_=oacc[:])
            else:
                nc.vector.tensor_copy(out=ot[:], in_=oacc[:])
            # ob shares the oacc pool banks: rotation interleaves
            # oacc(g) -> ob(g) -> oacc(g+1), each WAR-safe by then.
            # dv+1 padding keeps each transpose's PSUM write 4B-aligned.
            ob = oa_ps.tile([128, 4, dv + 1], BF16, tag="oacc")
            for u in range(4):
                nc.tensor.transpose(
                    ob[:, u, 0:dv],
                    ot[:, u * 128 : (u + 1) * 128],
                    ident_bf[0:dv, 0:dv],
                )
            rec = small.tile([128, 4], F32, tag="rec")
            nc.vector.reciprocal(out=rec[:], in_=ob[:, :, d])
            for u in range(4):
                t = half * 4 + u
                nc.vector.tensor_scalar_mul(
                    ostage[:, t, :], ob[:, u, 0:d], rec[:, u : u + 1]
                )
            # store each half as soon as it is normalized: halves the
            # final-store drain after the last compute
            nc.sync.dma_start(
                out=o_ap[:, half * 4 : (half + 1) * 4, n * d : (n + 1) * d],
                in_=ostage[:, half * 4 : (half + 1) * 4, :],
            )

        # software pipeline: mm2 slots run in PAIRS at J = pair*8 + 7 + s
        # (s even) — one 8-matmul mm2 burst every other jb halves the
        # mm1<->mm2 weight-switch overhead, and at most 2 oacc groups are
        # ever live.
        n_slots = heads // 2 * nt
        slot_at = {}
        for pair in range(heads // 2):
            for s in range(0, nt, 2):
                J = pair * nt + 7 + s
                slot_at.setdefault(J, []).extend(
                    (pair, si) for si in (s, s + 1)
                )
        last_J = heads // 2 * nt - 1
        states = []
        for pair in range(heads // 2):
            cur = load_and_transpose(pair)
            states.append(cur)
            for jb in range(nt):
                J = pair * nt + jb
                todo = slot_at.pop(J, [])
                mm1_exp(cur, jb)
                for p_, si in todo:
                    mm2_slot(states[p_], si)
                if not todo and J < 7:
                    # keep the PE clock gate warm through the fill phase
                    pe_filler(3)
        for J in sorted(slot_at):
            for p_, si in slot_at[J]:
                mm2_slot(states[p_], si, tail=True)

    _split_multi_waits(nc, mybir)
    return nc


def _get_nc():
    if "nc" not in _cache:
        _cache["nc"] = _build_nc()
    return _cache["nc"]


def _run(q, k, v, trace=False):
    from concourse.bass_utils import run_bass_kernel_spmd

    b, heads, h, w, d = 8, 8, 32, 32, 64
    q = np.ascontiguousarray(np.asarray(q, dtype=np.float32))
    k = np.ascontiguousarray(np.asarray(k, dtype=np.float32))
    v = np.ascontiguousarray(np.asarray(v, dtype=np.float32))
    assert q.shape == (b, heads, h, w, d), q.shape

    nc = _get_nc()
    in_maps = [
        {
            "q": q[c].reshape(heads, h * w, d),
            "k": k[c].reshape(heads, h * w, d),
            "v": v[c].reshape(heads, h * w, d),
        }
        for c in range(b)
    ]
    res = run_bass_kernel_spmd(nc, in_maps, core_ids=list(range(b)), trace=trace)
    out = np.stack(
        [res.results[c]["out"].reshape(h, w, heads * d) for c in range(b)]
    )
    return out, res


def kernel(q, k, v):
    out, _ = _run(q, k, v)
    return out



# revision 15
# speedup vs baseline: 1.1556x; 1.1556x over previous
"""Trainium2 Bass kernel for multi-head quadratic spatial attention.

Problem: q,k,v [b=8, heads=8, h=32, w=32, d=64] fp32; full attention over
the 1024-position spatial grid independently per (b, head); output
[b, h, w, heads*d].

Sharding: data-parallel over batch — core c handles b=c (8 heads of
[1024, 64] attention per core), no cross-core communication.

Per-core pipeline (heads processed in PAIRS; matmuls bf16 with fp32 PSUM
accumulation). The PE executes serially on this toolchain, so the design
minimizes streamed columns + instruction count and keeps the HAM clock
gate warm (no transpose-heavy stretches > ~3.4us, dummy-matmul warm-up):
  - p-major seq tiling (seq = p*8 + t); ONE 4D casting DMA per (tensor,
    pair) interleaving the two heads -> 3 gpsimd triggers per pair
  - 40 dummy ident matmuls warm the PE clock gate (1.2 -> 2.4 GHz) while
    the first DMAs land
  - pair-interleaved natural tiles [128, t, 2, d]: one [128,128] PE
    transpose per block yields head A's d-rows on partitions 0:64 and
    B's on 64:128 — the packed pair layout mm1 wants
  - mm1 row-tiled: head A contracts on PE rows 0:64, head B on 64:128
    -> St [128, 1024] fp32 (separate tiles, freed by their own exp)
  - exp on ScalarE (activation Exp); optional per-(jb, head) offload to
    VectorE via the Schraudolph bit-trick (fused tensor_scalar
    mult+add -> int16 == bf16 exp approx) to unload the ScalarE
  - mm2: lhsT = [V | 1] j-chunk [128, 65] bf16, rhs = Pt slices ->
    accumulate PSUM Ot [65, 512] per i-half; row 64 = softmax sums
  - epilogue in bf16: ot copy on VectorE, PE transposes back (FWL), one
    batched reciprocal [128,4] + per-block tensor_scalar normalize into
    fp32 ostage, stores on the sync HWDGE ring
"""

from contextlib import ExitStack

import numpy as np

F32 = None
BF16 = None
I16 = None

_cache = {}

# Schraudolph exp in bf16 bit-space: bf16_bits(exp(s*x)) ~= round(x*A + B)
# A = s * 2^7/ln2, B = 2^7*(127 - sigma), sigma = 0.0430 balances the
# piecewise-linear 2^frac error (max rel err ~3%, mostly cancelling in the
# softmax average).
SCALE = 64.0 ** -0.5
SCHRAUD_A = SCALE * 128.0 / float(np.log(2.0))
SCHRAUD_B = 128.0 * (127.0 - 0.0430)

N_WARM = 20  # dummy matmuls to flip the PE HAM clock gate before real work

# (jb, head-in-pair) St tiles exp'd on VectorE via the Schraudolph bit-trick
# instead of ScalarE's exact exp. Head B's tiles all go to VectorE: exp-A
# (ScalarE, ~1.15us) and exp-B (VectorE, ~1.19us) then run CONCURRENTLY, so
# stA/stB release near-simultaneously and the scheduler can bake the mm1
# quartet as A,B,A,B (adjacent disjoint row groups -> concurrent on the PE).
SCHRAUD_TILES = frozenset((jb, 1) for jb in range(8))


def _imports():
    global F32, BF16, I16
    import concourse.bass as bass
    import concourse.tile as tile
    from concourse import mybir
    from concourse.masks import make_identity

    F32 = mybir.dt.float32
    BF16 = mybir.dt.bfloat16
    I16 = mybir.dt.int16
    return bass, tile, mybir, make_identity


def _interleave_quartets(nc, mybir, quartets):
    """Post-schedule pass: rebake each mm1 quartet as A0,B0,A1,B1 in the PE
    stream (adjacent disjoint row groups stream CONCURRENTLY on the PE).

    The tile scheduler's cost model sees the DVE as backlogged and bakes the
    B matmuls several mm2 slots after the A's, which forfeits row-group
    concurrency.  On hardware the B's DVE wait is long satisfied by then, so
    we permute the PE stream directly: pull each B (with its LDWEIGHTS)
    forward to sit right after its A sibling.  All waits (any engine) on
    semaphores that PE instructions increment are then remapped: a wait for
    "first v PE-increments done" must now cover the same SET of original
    instructions at their new positions, i.e. new_v = prefix count at the
    maximum new position of that set.
    """
    pe_eng = mybir.EngineType.PE

    for f in nc.m.functions:
        for bb in f.blocks:
            insts = bb.instructions
            pe_idx = [i for i, ins in enumerate(insts) if ins.engine == pe_eng]
            if not pe_idx:
                continue
            stream = [insts[i] for i in pe_idx]
            name_pos = {ins.name: p for p, ins in enumerate(stream)}

            # unit = matmul + its immediately-preceding ldweights
            def unit(p):
                if p > 0 and type(stream[p - 1]).__name__ == "InstLdweights":
                    return [p - 1, p]
                return [p]

            order = list(range(len(stream)))
            for qa0, qb0, qa1, qb1 in quartets:
                if any(n not in name_pos for n in (qa0, qb0, qa1, qb1)):
                    continue
                pos = {n: order.index(name_pos[n]) for n in (qa0, qb0, qa1, qb1)}
                # desired: A0 < B0 < A1 < B1 contiguous at A0's slot
                taken = []
                for n in (qa0, qb0, qa1, qb1):
                    taken.extend(unit(name_pos[n]))
                taken_set = set(taken)
                anchor = min(order.index(t) for t in taken)
                rest = [x for x in order if x not in taken_set]
                new_unit_seq = []
                for n in (qa0, qb0, qa1, qb1):
                    new_unit_seq.extend(unit(name_pos[n]))
                order = rest[:anchor] + new_unit_seq + rest[anchor:]

            if order == list(range(len(stream))):
                continue

            new_stream = [stream[i] for i in order]
            # old position -> new position
            new_pos_of_old = [0] * len(stream)
            for newp, oldp in enumerate(order):
                new_pos_of_old[oldp] = newp

            # per-sem prefix increment arrays (old and new order)
            def upd_map(ins):
                out = {}
                si = ins.sync_info
                if si and si.on_update:
                    for u in si.on_update:
                        if u.sync_type == "semaphore":
                            out[u.id] = out.get(u.id, 0) + (u.update_value or 1)
                return out

            sem_ids = set()
            for ins in stream:
                sem_ids.update(upd_map(ins))
            old_prefix = {s: [0] * (len(stream) + 1) for s in sem_ids}
            new_prefix = {s: [0] * (len(stream) + 1) for s in sem_ids}
            for p, ins in enumerate(stream):
                um = upd_map(ins)
                for s in sem_ids:
                    old_prefix[s][p + 1] = old_prefix[s][p] + um.get(s, 0)
            for p, ins in enumerate(new_stream):
                um = upd_map(ins)
                for s in sem_ids:
                    new_prefix[s][p + 1] = new_prefix[s][p] + um.get(s, 0)

            def remap_wait(w):
                if w.sync_type != "semaphore" or w.id not in sem_ids:
                    return
                v = w.wait_value
                if v is None or v <= 0:
                    return
                op = old_prefix[w.id]
                if v > op[-1]:
                    return  # counts from a previous block epoch etc.
                # minimal k with old_prefix[k] >= v  -> set = old stream [0:k)
                import bisect
                k = bisect.bisect_left(op, v)
                max_new = max(new_pos_of_old[i] for i in range(k))
                w.wait_value = new_prefix[w.id][max_new + 1]

            for bb2 in f.blocks:
                for ins in bb2.instructions:
                    si = ins.sync_info
                    if si and si.on_wait:
                        for w in si.on_wait:
                            remap_wait(w)

            # write back: permuted PE stream into the same slots
            for slot, ins in zip(pe_idx, new_stream):
                insts[slot] = ins
            bb.instructions = insts


def _split_multi_waits(nc, mybir):
    """Walrus in this container supports only ONE sync-wait per instruction.
    Hoist extra waits onto same-engine InstNoOp's inserted just before."""
    ctr = 0
    for f in nc.m.functions:
        for bb in f.blocks:
            insts = bb.instructions
            if not any(
                i.sync_info and i.sync_info.on_wait and len(i.sync_info.on_wait) > 1
                for i in insts
            ):
                continue
            out = []
            for inst in insts:
                si = inst.sync_info
                waits = list(si.on_wait) if si and si.on_wait else []
                if len(waits) > 1:
                    for w in waits[:-1]:
                        ctr += 1
                        nop = mybir.InstNoOp(
                            name=f"I-wsplit-{ctr}",
                            engine=inst.engine,
                            ins=[],
                            outs=[],
                            sync_info=mybir.SyncInfo(on_wait=[w], on_update=[]),
                        )
                        nc.register_instruction(nop)
                        out.append(nop)
                    si.on_wait = waits[-1:]
                out.append(inst)
            bb.instructions = out


def _build_nc(heads=8, seq=1024, d=64):
    bass, tile, mybir, make_identity = _imports()
    assert heads % 2 == 0 and seq == 1024 and d == 64
    nt = seq // 128          # 8 blocks of 128 positions
    nh = seq // 512          # 2 i-halves of 512
    dv = d + 1
    TS_MULT = mybir.AluOpType.mult
    TS_ADD = mybir.AluOpType.add

    nc = bass.Bass(trn_type="TRN2", target_bir_lowering=False)
    quartets = []  # (A0, B0, A1, B1) matmul names for post-schedule interleave
    q_d = nc.dram_tensor("q", [heads, seq, d], F32, kind="ExternalInput")
    k_d = nc.dram_tensor("k", [heads, seq, d], F32, kind="ExternalInput")
    v_d = nc.dram_tensor("v", [heads, seq, d], F32, kind="ExternalInput")
    o_d = nc.dram_tensor("out", [seq, heads * d], F32, kind="ExternalOutput")

    # p-major: seq = p*nt + t; per-(p, t) HBM runs are 256B contiguous
    q_ap = q_d[:].rearrange("n (p t) d -> n p t d", p=128)
    k_ap = k_d[:].rearrange("n (p t) d -> n p t d", p=128)
    v_ap = v_d[:].rearrange("n (p t) d -> n p t d", p=128)
    o_ap = o_d[:].rearrange("(p t) c -> p t c", p=128)

    with tile.TileContext(nc) as tc, ExitStack() as ctx:
        consts = ctx.enter_context(tc.tile_pool(name="consts", bufs=1))
        nat = ctx.enter_context(tc.tile_pool(name="nat", bufs=2))
        dmaj = ctx.enter_context(tc.tile_pool(name="dmaj", bufs=2))
        ptp = ctx.enter_context(tc.tile_pool(name="ptp", bufs=36))
        otp = ctx.enter_context(tc.tile_pool(name="otp", bufs=3))
        outp = ctx.enter_context(tc.tile_pool(name="outp", bufs=3))
        small = ctx.enter_context(tc.tile_pool(name="small", bufs=4))

        # PSUM banks: st 2x2 (0-3) + oacc/ob/warm 2x1 (4-5) + tp 2x1 (6-7)
        st_ps = ctx.enter_context(tc.tile_pool(name="st_ps", bufs=2, space="PSUM"))
        oa_ps = ctx.enter_context(tc.tile_pool(name="oa_ps", bufs=2, space="PSUM"))
        tp_ps = ctx.enter_context(tc.tile_pool(name="tp_ps", bufs=2, space="PSUM"))

        ident_bf = consts.tile([128, 128], BF16)
        make_identity(nc, ident_bf[:])
        ident_f32 = consts.tile([128, 128], F32)
        make_identity(nc, ident_f32[:])

        # Warm-up / filler matmuls keep the PE HAM clock gate at 2.4 GHz:
        # an idle (or transpose-only) stretch > ~3.4us re-throttles the PE
        # clock to 1.2 GHz for the next several microseconds. wsrc is
        # memset-ready within ~200ns of kernel start. N=512 streams give
        # ~213ns of HAM-counted busy per filler instruction.
        wsrc = consts.tile([128, 512], BF16)
        nc.vector.memset(wsrc[:], 0.25)

        def pe_filler(n):
            # fresh tile per burst: fillers WAR-chain only onto transient
            # transpose tiles, never onto live oacc accumulators
            t = tp_ps.tile([128, 512], F32, tag="tp", name="warm")
            for _ in range(n):
                nc.tensor.matmul(
                    t[:], wsrc[:, 0:128], wsrc[:], start=True, stop=True
                )

        pe_filler(N_WARM)

        def load_and_transpose(pair):
            """DMA pair inputs (bf16 cast, one 4D DMA per tensor) and build
            packed d-major tiles: head A on partitions 0:64, head B on
            64:128 (one [128,128] PE transpose per block)."""
            st8 = {"heads": (2 * pair, 2 * pair + 1), "v": None, "pts": {},
                   "oacc": {}, "ostage": {}}
            # pair-interleaved natural tiles: [..., 2, d] with head A at
            # index 0 and head B at 1, so one [128, 128] PE transpose of a
            # block yields A's d-rows on partitions 0:64 and B's on 64:128.
            qp = nat.tile([128, nt, 2, d], BF16, tag="qp")
            kp = nat.tile([128, nt, 2, d], BF16, tag="kp")
            hh = nt // 2
            if pair == 0:
                # halved loads, first halves of BOTH heads first, so the
                # first transpose group can start after ~2 trigger slots
                for lo, hi in ((0, hh), (hh, nt)):
                    for src_ap, dst in ((q_ap, qp), (k_ap, kp)):
                        for idx, n in enumerate(st8["heads"]):
                            nc.gpsimd.dma_start(
                                out=dst[:, lo:hi, idx, :], in_=src_ap[n, :, lo:hi]
                            )
            else:
                for idx, n in enumerate(st8["heads"]):
                    nc.gpsimd.dma_start(out=qp[:, :, idx, :], in_=q_ap[n])
                    nc.gpsimd.dma_start(out=kp[:, :, idx, :], in_=k_ap[n])
            vp = nat.tile([128, nt, 2, dv], BF16, tag="vp")
            # ones columns for the softmax-denominator trick
            nc.vector.memset(vp[:, :, :, d : d + 1], 1.0)
            for idx, n in enumerate(st8["heads"]):
                nc.gpsimd.dma_start(out=vp[:, :, idx, 0:d], in_=v_ap[n])
            st8["v"] = vp
            qt = dmaj.tile([128, seq], BF16, tag="qt")
            kt = dmaj.tile([128, seq], BF16, tag="kt")
            for g in range(nt // 4):
                for src, dst in ((qp, qt), (kp, kt)):
                    tp = tp_ps.tile([128, 512], BF16, tag="tp")
                    for u in range(4):
                        t = g * 4 + u
                        nc.tensor.transpose(
                            tp[:, u * 128 : (u + 1) * 128],
                            src[:, t, :, :],
                            ident_bf[:],
                        )
                    nc.vector.tensor_copy(
                        out=dst[:, g * 512 : (g + 1) * 512], in_=tp[:]
                    )
                    if pair == 0:
                        # PE is otherwise DMA-bound here; keep the clock warm
                        pe_filler(1)
            st8["qt"], st8["kt"] = qt, kt
            return st8

        def _exp(s, jb, idx, st):
            """Evacuate one St tile: exact Exp on ScalarE, or the Schraudolph
            bit-trick on VectorE for tiles in SCHRAUD_TILES.  High priority:
            St must evacuate ASAP to release PSUM for the next mm1 quartet,
            ahead of same-engine epilogue work (ot copies / normalize)."""
            with tc.high_priority(offset=30):
                pt = ptp.tile([128, seq], BF16, name="pt", tag="pt")
                if (jb, idx) in SCHRAUD_TILES:
                    nc.vector.tensor_scalar(
                        out=pt[:].bitcast(I16),
                        in0=st[:],
                        scalar1=SCHRAUD_A,
                        scalar2=SCHRAUD_B,
                        op0=TS_MULT,
                        op1=TS_ADD,
                    )
                else:
                    nc.scalar.activation(
                        out=pt[:],
                        in_=st[:],
                        func=mybir.ActivationFunctionType.Exp,
                        scale=SCALE,
                    )
                s["pts"][(jb, idx)] = pt

        def mm1_exp(s, jb):
            """One software-pipelined mm1 step: head A's block jb together
            with head B's block jb-1 (B SHIFTED ONE STEP behind A).

            Head A contracts on PE rows 0:64 (row group h0), head B on
            64:128 (h64).  Emitted interleaved A(c),B(c): consecutive
            instructions target DISJOINT row groups, so the PE streams them
            CONCURRENTLY (~2x issue rate vs same-group runs).  The one-step
            B shift is what makes this robust: B(jb-1)'s PSUM slot was
            released by exp-B(jb-2), a full step ago, so whenever A(jb)
            becomes ready B is ready too and the scheduler bakes the
            quartet adjacently instead of splitting it around mm2 work."""
            qt, kt = s["qt"], s["kt"]
            stA = st_ps.tile([128, seq], F32, name="stA", tag="st")
            stB = None
            if jb > 0:
                stB = st_ps.tile([128, seq], F32, name="stB", tag="st")
            names = []
            for c in range(nh):
                names.append(nc.tensor.matmul(
                    stA[:, c * 512 : (c + 1) * 512],
                    kt[0:64, jb * 128 : (jb + 1) * 128],
                    qt[0:64, c * 512 : (c + 1) * 512],
                    start=True,
                    stop=True,
                ).ins.name)
                if stB is not None:
                    names.append(nc.tensor.matmul(
                        stB[:, c * 512 : (c + 1) * 512],
                        kt[64:128, (jb - 1) * 128 : jb * 128],
                        qt[64:128, c * 512 : (c + 1) * 512],
                        start=True,
                        stop=True,
                    ).ins.name)
            if stB is not None:
                quartets.append(tuple(names))  # (A0, B0, A1, B1)
            _exp(s, jb, 0, stA)
            if stB is not None:
                _exp(s, jb - 1, 1, stB)

        def mm1_tail(s):
            """Head B's last block (jb=nt-1), deferred by the one-step
            shift."""
            qt, kt = s["qt"], s["kt"]
            stB = st_ps.tile([128, seq], F32, name="stB", tag="st")
            for c in range(nh):
                nc.tensor.matmul(
                    stB[:, c * 512 : (c + 1) * 512],
                    kt[64:128, (nt - 1) * 128 : nt * 128],
                    qt[64:128, c * 512 : (c + 1) * 512],
                    start=True,
                    stop=True,
                )
            _exp(s, nt - 1, 1, stB)

        # slot s -> (group, phase); phase-1 slots sit at s>=2 so they only
        # run once all 8 jbs' Pt tiles exist (6-jb-shifted pipeline), while
        # at most 2 groups' oacc accumulators are ever live.
        SLOT_ORDER = [(0, 0), (1, 0), (0, 1), (1, 1), (2, 0), (3, 0), (2, 1), (3, 1)]

        def mm2_slot(s, slot, tail=False):
            """One PE-stream slot of the pair's mm2: 4 accumulating
            matmuls of a (head, half) group; epilogue on the closing
            phase."""
            g, phase = SLOT_ORDER[slot]
            idx, half = g // 2, g % 2
            if phase == 0:
                s["oacc"][g] = oa_ps.tile([dv, 512], F32, name="oacc", tag="oacc")
            oacc = s["oacc"][g]
            off = half * 512
            for jj in range(4):
                jb = phase * 4 + jj
                pt = s["pts"][(jb, idx)]
                nc.tensor.matmul(
                    oacc[:],
                    s["v"][:, jb, idx, :],
                    pt[:, off : off + 512],
                    start=(jb == 0),
                    stop=(jb == nt - 1),
                )
            if phase == 1:
                _epilogue(s, idx, half, oacc, tail=tail)

        def _epilogue(s, idx, half, oacc, tail=False):
            n = s["heads"][idx]
            if idx not in s["ostage"]:
                s["ostage"][idx] = outp.tile(
                    [128, nt, d], F32, name="ostage", tag="ostage"
                )
            ostage = s["ostage"][idx]
            ot = otp.tile([dv, 512], BF16, tag="ot")
            # ScalarE owns all ot evacuations: with exp-B moved to VectorE,
            # ScalarE (exp-A + ot) and VectorE (exp-B + normalize/recip)
            # carry ~equal load, and VectorE stays off the stB release path.
            nc.scalar.copy(out=ot[:], in_=oacc[:])
            # ob shares the oacc pool banks: rotation interleaves
            # oacc(g) -> ob(g) -> oacc(g+1), each WAR-safe by then.
            # dv+1 padding keeps each transpose's PSUM write 4B-aligned.
            ob = oa_ps.tile([128, 4, dv + 1], BF16, tag="oacc")
            for u in range(4):
                nc.tensor.transpose(
                    ob[:, u, 0:dv],
                    ot[:, u * 128 : (u + 1) * 128],
                    ident_bf[0:dv, 0:dv],
                )
            rec = small.tile([128, 4], F32, tag="rec")
            nc.vector.reciprocal(out=rec[:], in_=ob[:, :, d])
            # single batched normalize: rec broadcast along d via stride-0 AP
            nc.vector.tensor_mul(
                ostage[:, half * 4 : (half + 1) * 4, :],
                ob[:, :, 0:d],
                rec[:, :, None].broadcast_to([128, 4, d]),
            )
            # store each half as soon as it is normalized: halves the
            # final-store drain after the last compute
            nc.sync.dma_start(
                out=o_ap[:, half * 4 : (half + 1) * 4, n * d : (n + 1) * d],
                in_=ostage[:, half * 4 : (half + 1) * 4, :],
            )

        # software pipeline: mm2 slots run in PAIRS at J = pair*8 + 7 + s
        # (s even) — one 8-matmul mm2 burst every other jb halves the
        # mm1<->mm2 weight-switch overhead, and at most 2 oacc groups are
        # ever live.
        n_slots = heads // 2 * nt
        slot_at = {}
        for pair in range(heads // 2):
            for s in range(0, nt, 2):
                J = pair * nt + 7 + s
                slot_at.setdefault(J, []).extend(
                    (pair, si) for si in (s, s + 1)
                )
        last_J = heads // 2 * nt - 1
        states = []
        for pair in range(heads // 2):
            cur = load_and_transpose(pair)
            states.append(cur)
            for jb in range(nt):
                J = pair * nt + jb
                todo = slot_at.pop(J, [])
                # mm2 burst FIRST: PE work between exp(jb-1) and mm1(jb) so
                # the St tiles are free when the mm1 quartet issues.
                for p_, si in todo:
                    mm2_slot(states[p_], si)
                mm1_exp(cur, jb)
                if not todo and J < 7:
                    # keep the PE clock gate warm through the fill phase
                    pe_filler(3)
            mm1_tail(cur)
        for J in sorted(slot_at):
            for p_, si in slot_at[J]:
                mm2_slot(states[p_], si, tail=True)

    _interleave_quartets(nc, mybir, quartets)
    _split_multi_waits(nc, mybir)
    return nc


def _get_nc():
    if "nc" not in _cache:
        _cache["nc"] = _build_nc()
    return _cache["nc"]


def _run(q, k, v, trace=False):
    from concourse.bass_utils import run_bass_kernel_spmd

    b, heads, h, w, d = 8, 8, 32, 32, 64
    q = np.ascontiguousarray(np.asarray(q, dtype=np.float32))
    k = np.ascontiguousarray(np.asarray(k, dtype=np.float32))
    v = np.ascontiguousarray(np.asarray(v, dtype=np.float32))
    assert q.shape == (b, heads, h, w, d), q.shape

    nc = _get_nc()
    in_maps = [
        {
            "q": q[c].reshape(heads, h * w, d),
            "k": k[c].reshape(heads, h * w, d),
            "v": v[c].reshape(heads, h * w, d),
        }
        for c in range(b)
    ]
    res = run_bass_kernel_spmd(nc, in_maps, core_ids=list(range(b)), trace=trace)
    out = np.stack(
        [res.results[c]["out"].reshape(h, w, heads * d) for c in range(b)]
    )
    return out, res


def kernel(q, k, v):
    out, _ = _run(q, k, v)
    return out



# revision 20
# speedup vs baseline: 1.1580x; 1.0020x over previous
"""Trainium2 Bass kernel for multi-head quadratic spatial attention.

Problem: q,k,v [b=8, heads=8, h=32, w=32, d=64] fp32; full attention over
the 1024-position spatial grid independently per (b, head); output
[b, h, w, heads*d].

Sharding: data-parallel over batch — core c handles b=c (8 heads of
[1024, 64] attention per core), no cross-core communication.

Per-core pipeline (heads processed in PAIRS; matmuls bf16 with fp32 PSUM
accumulation). The PE executes serially on this toolchain, so the design
minimizes streamed columns + instruction count and keeps the HAM clock
gate warm (no transpose-heavy stretches > ~3.4us, dummy-matmul warm-up):
  - p-major seq tiling (seq = p*8 + t); ONE 4D casting DMA per (tensor,
    pair) interleaving the two heads -> 3 gpsimd triggers per pair
  - 40 dummy ident matmuls warm the PE clock gate (1.2 -> 2.4 GHz) while
    the first DMAs land
  - pair-interleaved natural tiles [128, t, 2, d]: one [128,128] PE
    transpose per block yields head A's d-rows on partitions 0:64 and
    B's on 64:128 — the packed pair layout mm1 wants
  - mm1 row-tiled: head A contracts on PE rows 0:64, head B on 64:128
    -> St [128, 1024] fp32 (separate tiles, freed by their own exp)
  - exp on ScalarE (activation Exp); optional per-(jb, head) offload to
    VectorE via the Schraudolph bit-trick (fused tensor_scalar
    mult+add -> int16 == bf16 exp approx) to unload the ScalarE
  - mm2: lhsT = [V | 1] j-chunk [128, 65] bf16, rhs = Pt slices ->
    accumulate PSUM Ot [65, 512] per i-half; row 64 = softmax sums
  - epilogue in bf16: ot copy on VectorE, PE transposes back (FWL), one
    batched reciprocal [128,4] + per-block tensor_scalar normalize into
    fp32 ostage, stores on the sync HWDGE ring
"""

from contextlib import ExitStack

import numpy as np

F32 = None
BF16 = None
I16 = None

_cache = {}

# Schraudolph exp in bf16 bit-space: bf16_bits(exp(s*x)) ~= round(x*A + B)
# A = s * 2^7/ln2, B = 2^7*(127 - sigma), sigma = 0.0430 balances the
# piecewise-linear 2^frac error (max rel err ~3%, mostly cancelling in the
# softmax average).
SCALE = 64.0 ** -0.5
SCHRAUD_A = SCALE * 128.0 / float(np.log(2.0))
SCHRAUD_B = 128.0 * (127.0 - 0.0430)

N_WARM = 20  # dummy matmuls to flip the PE HAM clock gate before real work

# (jb, head-in-pair) St tiles exp'd on VectorE via the Schraudolph bit-trick
# instead of ScalarE's exact exp. Head B's tiles all go to VectorE: exp-A
# (ScalarE, ~1.15us) and exp-B (VectorE, ~1.19us) then run CONCURRENTLY, so
# stA/stB release near-simultaneously and the scheduler can bake the mm1
# quartet as A,B,A,B (adjacent disjoint row groups -> concurrent on the PE).
SCHRAUD_TILES = frozenset((jb, 1) for jb in range(8))


def _imports():
    global F32, BF16, I16
    import concourse.bass as bass
    import concourse.tile as tile
    from concourse import mybir
    from concourse.masks import make_identity

    F32 = mybir.dt.float32
    BF16 = mybir.dt.bfloat16
    I16 = mybir.dt.int16
    return bass, tile, mybir, make_identity


def _interleave_quartets(nc, mybir, quartets):
    """Post-schedule pass: rebake each mm1 quartet as A0,B0,A1,B1 in the PE
    stream (adjacent disjoint row groups stream CONCURRENTLY on the PE).

    The tile scheduler's cost model sees the DVE as backlogged and bakes the
    B matmuls several mm2 slots after the A's, which forfeits row-group
    concurrency.  On hardware the B's DVE wait is long satisfied by then, so
    we permute the PE stream directly: pull each B (with its LDWEIGHTS)
    forward to sit right after its A sibling.  All waits (any engine) on
    semaphores that PE instructions increment are then remapped: a wait for
    "first v PE-increments done" must now cover the same SET of original
    instructions at their new positions, i.e. new_v = prefix count at the
    maximum new position of that set.
    """
    pe_eng = mybir.EngineType.PE

    for f in nc.m.functions:
        for bb in f.blocks:
            insts = bb.instructions
            pe_idx = [i for i, ins in enumerate(insts) if ins.engine == pe_eng]
            if not pe_idx:
                continue
            stream = [insts[i] for i in pe_idx]
            name_pos = {ins.name: p for p, ins in enumerate(stream)}

            # unit = matmul + its immediately-preceding ldweights
            def unit(p):
                if p > 0 and type(stream[p - 1]).__name__ == "InstLdweights":
                    return [p - 1, p]
                return [p]

            order = list(range(len(stream)))
            for qa0, qb0, qa1, qb1 in quartets:
                if any(n not in name_pos for n in (qa0, qb0, qa1, qb1)):
                    continue
                pos = {n: order.index(name_pos[n]) for n in (qa0, qb0, qa1, qb1)}
                # desired: A0 < B0 < A1 < B1 contiguous at A0's slot
                taken = []
                for n in (qa0, qb0, qa1, qb1):
                    taken.extend(unit(name_pos[n]))
                taken_set = set(taken)
                anchor = min(order.index(t) for t in taken)
                rest = [x for x in order if x not in taken_set]
                new_unit_seq = []
                for n in (qa0, qb0, qa1, qb1):
                    new_unit_seq.extend(unit(name_pos[n]))
                order = rest[:anchor] + new_unit_seq + rest[anchor:]

            if order == list(range(len(stream))):
                continue

            new_stream = [stream[i] for i in order]
            # old position -> new position
            new_pos_of_old = [0] * len(stream)
            for newp, oldp in enumerate(order):
                new_pos_of_old[oldp] = newp

            # per-sem prefix increment arrays (old and new order)
            def upd_map(ins):
                out = {}
                si = ins.sync_info
                if si and si.on_update:
                    for u in si.on_update:
                        if u.sync_type == "semaphore":
                            out[u.id] = out.get(u.id, 0) + (u.update_value or 1)
                return out

            sem_ids = set()
            for ins in stream:
                sem_ids.update(upd_map(ins))
            old_prefix = {s: [0] * (len(stream) + 1) for s in sem_ids}
            new_prefix = {s: [0] * (len(stream) + 1) for s in sem_ids}
            for p, ins in enumerate(stream):
                um = upd_map(ins)
                for s in sem_ids:
                    old_prefix[s][p + 1] = old_prefix[s][p] + um.get(s, 0)
            for p, ins in enumerate(new_stream):
                um = upd_map(ins)
                for s in sem_ids:
                    new_prefix[s][p + 1] = new_prefix[s][p] + um.get(s, 0)

            def remap_wait(w):
                if w.sync_type != "semaphore" or w.id not in sem_ids:
                    return
                v = w.wait_value
                if v is None or v <= 0:
                    return
                op = old_prefix[w.id]
                if v > op[-1]:
                    return  # counts from a previous block epoch etc.
                # minimal k with old_prefix[k] >= v  -> set = old stream [0:k)
                import bisect
                k = bisect.bisect_left(op, v)
                max_new = max(new_pos_of_old[i] for i in range(k))
                w.wait_value = new_prefix[w.id][max_new + 1]

            for bb2 in f.blocks:
                for ins in bb2.instructions:
                    si = ins.sync_info
                    if si and si.on_wait:
                        for w in si.on_wait:
                            remap_wait(w)

            # write back: permuted PE stream into the same slots
            for slot, ins in zip(pe_idx, new_stream):
                insts[slot] = ins
            bb.instructions = insts


def _split_multi_waits(nc, mybir):
    """Walrus in this container supports only ONE sync-wait per instruction.
    Hoist extra waits onto same-engine InstNoOp's inserted just before."""
    ctr = 0
    for f in nc.m.functions:
        for bb in f.blocks:
            insts = bb.instructions
            if not any(
                i.sync_info and i.sync_info.on_wait and len(i.sync_info.on_wait) > 1
                for i in insts
            ):
                continue
            out = []
            for inst in insts:
                si = inst.sync_info
                waits = list(si.on_wait) if si and si.on_wait else []
                if len(waits) > 1:
                    for w in waits[:-1]:
                        ctr += 1
                        nop = mybir.InstNoOp(
                            name=f"I-wsplit-{ctr}",
                            engine=inst.engine,
                            ins=[],
                            outs=[],
                            sync_info=mybir.SyncInfo(on_wait=[w], on_update=[]),
                        )
                        nc.register_instruction(nop)
                        out.append(nop)
                    si.on_wait = waits[-1:]
                out.append(inst)
            bb.instructions = out


def _build_nc(heads=8, seq=1024, d=64):
    bass, tile, mybir, make_identity = _imports()
    assert heads % 2 == 0 and seq == 1024 and d == 64
    nt = seq // 128          # 8 blocks of 128 positions
    nh = seq // 512          # 2 i-halves of 512
    dv = d + 1
    TS_MULT = mybir.AluOpType.mult
    TS_ADD = mybir.AluOpType.add

    nc = bass.Bass(trn_type="TRN2", target_bir_lowering=False)
    quartets = []  # (A0, B0, A1, B1) matmul names for post-schedule interleave
    q_d = nc.dram_tensor("q", [heads, seq, d], F32, kind="ExternalInput")
    k_d = nc.dram_tensor("k", [heads, seq, d], F32, kind="ExternalInput")
    v_d = nc.dram_tensor("v", [heads, seq, d], F32, kind="ExternalInput")
    o_d = nc.dram_tensor("out", [seq, heads * d], F32, kind="ExternalOutput")

    # p-major: seq = p*nt + t; per-(p, t) HBM runs are 256B contiguous
    q_ap = q_d[:].rearrange("n (p t) d -> n p t d", p=128)
    k_ap = k_d[:].rearrange("n (p t) d -> n p t d", p=128)
    v_ap = v_d[:].rearrange("n (p t) d -> n p t d", p=128)
    o_ap = o_d[:].rearrange("(p t) c -> p t c", p=128)

    with tile.TileContext(nc) as tc, ExitStack() as ctx:
        consts = ctx.enter_context(tc.tile_pool(name="consts", bufs=1))
        nat = ctx.enter_context(tc.tile_pool(name="nat", bufs=2))
        dmaj = ctx.enter_context(tc.tile_pool(name="dmaj", bufs=2))
        ptp = ctx.enter_context(tc.tile_pool(name="ptp", bufs=36))
        otp = ctx.enter_context(tc.tile_pool(name="otp", bufs=3))
        outp = ctx.enter_context(tc.tile_pool(name="outp", bufs=3))
        small = ctx.enter_context(tc.tile_pool(name="small", bufs=4))

        # PSUM banks: st 2x2 (0-3) + oacc/ob/warm 2x1 (4-5) + tp 2x1 (6-7)
        st_ps = ctx.enter_context(tc.tile_pool(name="st_ps", bufs=2, space="PSUM"))
        oa_ps = ctx.enter_context(tc.tile_pool(name="oa_ps", bufs=2, space="PSUM"))
        tp_ps = ctx.enter_context(tc.tile_pool(name="tp_ps", bufs=2, space="PSUM"))

        ident_bf = consts.tile([128, 128], BF16)
        make_identity(nc, ident_bf[:])
        ident_f32 = consts.tile([128, 128], F32)
        make_identity(nc, ident_f32[:])

        # Warm-up / filler matmuls keep the PE HAM clock gate at 2.4 GHz:
        # an idle (or transpose-only) stretch > ~3.4us re-throttles the PE
        # clock to 1.2 GHz for the next several microseconds. wsrc is
        # memset-ready within ~200ns of kernel start. N=512 streams give
        # ~213ns of HAM-counted busy per filler instruction.
        wsrc = consts.tile([128, 512], BF16)
        nc.vector.memset(wsrc[:], 0.25)

        def pe_filler(n):
            # fresh tile per burst: fillers WAR-chain only onto transient
            # transpose tiles, never onto live oacc accumulators
            t = tp_ps.tile([128, 512], F32, tag="tp", name="warm")
            for _ in range(n):
                nc.tensor.matmul(
                    t[:], wsrc[:, 0:128], wsrc[:], start=True, stop=True
                )

        pe_filler(N_WARM)

        def load_and_transpose(pair):
            """DMA pair inputs (bf16 cast, one 4D DMA per tensor) and build
            packed d-major tiles: head A on partitions 0:64, head B on
            64:128 (one [128,128] PE transpose per block)."""
            st8 = {"heads": (2 * pair, 2 * pair + 1), "v": None, "pts": {},
                   "oacc": {}, "ostage": {}}
            # pair-interleaved natural tiles: [..., 2, d] with head A at
            # index 0 and head B at 1, so one [128, 128] PE transpose of a
            # block yields A's d-rows on partitions 0:64 and B's on 64:128.
            qp = nat.tile([128, nt, 2, d], BF16, tag="qp")
            kp = nat.tile([128, nt, 2, d], BF16, tag="kp")
            hh = nt // 2
            if pair == 0:
                # halved loads, first halves of BOTH heads first, so the
                # first transpose group can start after ~2 trigger slots
                for lo, hi in ((0, hh), (hh, nt)):
                    for src_ap, dst in ((q_ap, qp), (k_ap, kp)):
                        for idx, n in enumerate(st8["heads"]):
                            nc.gpsimd.dma_start(
                                out=dst[:, lo:hi, idx, :], in_=src_ap[n, :, lo:hi]
                            )
            else:
                for idx, n in enumerate(st8["heads"]):
                    nc.gpsimd.dma_start(out=qp[:, :, idx, :], in_=q_ap[n])
                    nc.gpsimd.dma_start(out=kp[:, :, idx, :], in_=k_ap[n])
            vp = nat.tile([128, nt, 2, dv], BF16, tag="vp")
            # ones columns for the softmax-denominator trick
            nc.vector.memset(vp[:, :, :, d : d + 1], 1.0)
            for idx, n in enumerate(st8["heads"]):
                nc.gpsimd.dma_start(out=vp[:, :, idx, 0:d], in_=v_ap[n])
            st8["v"] = vp
            qt = dmaj.tile([128, seq], BF16, tag="qt")
            kt = dmaj.tile([128, seq], BF16, tag="kt")
            for g in range(nt // 4):
                for src, dst in ((qp, qt), (kp, kt)):
                    tp = tp_ps.tile([128, 512], BF16, tag="tp")
                    for u in range(4):
                        t = g * 4 + u
                        nc.tensor.transpose(
                            tp[:, u * 128 : (u + 1) * 128],
                            src[:, t, :, :],
                            ident_bf[:],
                        )
                    nc.vector.tensor_copy(
                        out=dst[:, g * 512 : (g + 1) * 512], in_=tp[:]
                    )
                    if pair == 0:
                        # PE is otherwise DMA-bound here; keep the clock warm
                        pe_filler(3)
            st8["qt"], st8["kt"] = qt, kt
            return st8

        def _exp(s, jb, idx, st):
            """Evacuate one St tile: exact Exp on ScalarE, or the Schraudolph
            bit-trick on VectorE for tiles in SCHRAUD_TILES.  High priority:
            St must evacuate ASAP to release PSUM for the next mm1 quartet,
            ahead of same-engine epilogue work (ot copies / normalize)."""
            with tc.high_priority(offset=30):
                pt = ptp.tile([128, seq], BF16, name="pt", tag="pt")
                if (jb, idx) in SCHRAUD_TILES:
                    nc.vector.tensor_scalar(
                        out=pt[:].bitcast(I16),
                        in0=st[:],
                        scalar1=SCHRAUD_A,
                        scalar2=SCHRAUD_B,
                        op0=TS_MULT,
                        op1=TS_ADD,
                    )
                else:
                    nc.scalar.activation(
                        out=pt[:],
                        in_=st[:],
                        func=mybir.ActivationFunctionType.Exp,
                        scale=SCALE,
                    )
                s["pts"][(jb, idx)] = pt

        def mm1_exp(s, jb):
            """One software-pipelined mm1 step: head A's block jb together
            with head B's block jb-1 (B SHIFTED ONE STEP behind A).

            Head A contracts on PE rows 0:64 (row group h0), head B on
            64:128 (h64).  Emitted interleaved A(c),B(c): consecutive
            instructions target DISJOINT row groups, so the PE streams them
            CONCURRENTLY (~2x issue rate vs same-group runs).  The one-step
            B shift is what makes this robust: B(jb-1)'s PSUM slot was
            released by exp-B(jb-2), a full step ago, so whenever A(jb)
            becomes ready B is ready too and the scheduler bakes the
            quartet adjacently instead of splitting it around mm2 work."""
            qt, kt = s["qt"], s["kt"]
            stA = st_ps.tile([128, seq], F32, name="stA", tag="st")
            stB = None
            if jb > 0:
                stB = st_ps.tile([128, seq], F32, name="stB", tag="st")
            names = []
            for c in range(nh):
                names.append(nc.tensor.matmul(
                    stA[:, c * 512 : (c + 1) * 512],
                    kt[0:64, jb * 128 : (jb + 1) * 128],
                    qt[0:64, c * 512 : (c + 1) * 512],
                    start=True,
                    stop=True,
                ).ins.name)
                if stB is not None:
                    names.append(nc.tensor.matmul(
                        stB[:, c * 512 : (c + 1) * 512],
                        kt[64:128, (jb - 1) * 128 : jb * 128],
                        qt[64:128, c * 512 : (c + 1) * 512],
                        start=True,
                        stop=True,
                    ).ins.name)
            if stB is not None:
                quartets.append(tuple(names))  # (A0, B0, A1, B1)
            _exp(s, jb, 0, stA)
            if stB is not None:
                _exp(s, jb - 1, 1, stB)

        def mm1_tail(s):
            """Head B's last block (jb=nt-1), deferred by the one-step
            shift."""
            qt, kt = s["qt"], s["kt"]
            stB = st_ps.tile([128, seq], F32, name="stB", tag="st")
            for c in range(nh):
                nc.tensor.matmul(
                    stB[:, c * 512 : (c + 1) * 512],
                    kt[64:128, (nt - 1) * 128 : nt * 128],
                    qt[64:128, c * 512 : (c + 1) * 512],
                    start=True,
                    stop=True,
                )
            _exp(s, nt - 1, 1, stB)

        # slot s -> (group, phase); phase-1 slots sit at s>=2 so they only
        # run once all 8 jbs' Pt tiles exist (6-jb-shifted pipeline), while
        # at most 2 groups' oacc accumulators are ever live.
        SLOT_ORDER = [(0, 0), (1, 0), (0, 1), (1, 1), (2, 0), (3, 0), (2, 1), (3, 1)]

        def mm2_slot(s, slot, tail=False):
            """One PE-stream slot of the pair's mm2: 4 accumulating
            matmuls of a (head, half) group; epilogue on the closing
            phase."""
            g, phase = SLOT_ORDER[slot]
            idx, half = g // 2, g % 2
            if phase == 0:
                s["oacc"][g] = oa_ps.tile([dv, 512], F32, name="oacc", tag="oacc")
            oacc = s["oacc"][g]
            off = half * 512
            for jj in range(4):
                jb = phase * 4 + jj
                pt = s["pts"][(jb, idx)]
                nc.tensor.matmul(
                    oacc[:],
                    s["v"][:, jb, idx, :],
                    pt[:, off : off + 512],
                    start=(jb == 0),
                    stop=(jb == nt - 1),
                )
            if phase == 1:
                _epilogue(s, idx, half, oacc, tail=tail)

        def _epilogue(s, idx, half, oacc, tail=False):
            n = s["heads"][idx]
            if idx not in s["ostage"]:
                s["ostage"][idx] = outp.tile(
                    [128, nt, d], F32, name="ostage", tag="ostage"
                )
            ostage = s["ostage"][idx]
            ot = otp.tile([dv, 512], BF16, tag="ot")
            # ScalarE owns all ot evacuations: with exp-B moved to VectorE,
            # ScalarE (exp-A + ot) and VectorE (exp-B + normalize/recip)
            # carry ~equal load, and VectorE stays off the stB release path.
            nc.scalar.copy(out=ot[:], in_=oacc[:])
            # ob shares the oacc pool banks: rotation interleaves
            # oacc(g) -> ob(g) -> oacc(g+1), each WAR-safe by then.
            # dv+1 padding keeps each transpose's PSUM write 4B-aligned.
            ob = oa_ps.tile([128, 4, dv + 1], BF16, tag="oacc")
            for u in range(4):
                nc.tensor.transpose(
                    ob[:, u, 0:dv],
                    ot[:, u * 128 : (u + 1) * 128],
                    ident_bf[0:dv, 0:dv],
                )
            rec = small.tile([128, 4], F32, tag="rec")
            nc.vector.reciprocal(out=rec[:], in_=ob[:, :, d])
            # single batched normalize: rec broadcast along d via stride-0 AP
            nc.vector.tensor_mul(
                ostage[:, half * 4 : (half + 1) * 4, :],
                ob[:, :, 0:d],
                rec[:, :, None].broadcast_to([128, 4, d]),
            )
            # store each half as soon as it is normalized: halves the
            # final-store drain after the last compute
            nc.sync.dma_start(
                out=o_ap[:, half * 4 : (half + 1) * 4, n * d : (n + 1) * d],
                in_=ostage[:, half * 4 : (half + 1) * 4, :],
            )

        # software pipeline: mm2 slots run in PAIRS at J = pair*8 + 7 + s
        # (s even) — one 8-matmul mm2 burst every other jb halves the
        # mm1<->mm2 weight-switch overhead, and at most 2 oacc groups are
        # ever live.
        n_slots = heads // 2 * nt
        slot_at = {}
        for pair in range(heads // 2):
            for s in range(0, nt, 2):
                J = pair * nt + 7 + s
                slot_at.setdefault(J, []).extend(
                    (pair, si) for si in (s, s + 1)
                )
        last_J = heads // 2 * nt - 1
        states = []
        for pair in range(heads // 2):
            cur = load_and_transpose(pair)
            states.append(cur)
            for jb in range(nt):
                J = pair * nt + jb
                todo = slot_at.pop(J, [])
                # mm2 burst FIRST: PE work between exp(jb-1) and mm1(jb) so
                # the St tiles are free when the mm1 quartet issues.
                for p_, si in todo:
                    mm2_slot(states[p_], si)
                mm1_exp(cur, jb)
                if not todo and J < 7:
                    # keep the PE clock gate warm through the fill phase
                    pe_filler(3)
            mm1_tail(cur)
        for J in sorted(slot_at):
            for p_, si in slot_at[J]:
                mm2_slot(states[p_], si, tail=True)

    _interleave_quartets(nc, mybir, quartets)
    _split_multi_waits(nc, mybir)
    return nc


def _get_nc():
    if "nc" not in _cache:
        _cache["nc"] = _build_nc()
    return _cache["nc"]


def _run(q, k, v, trace=False):
    from concourse.bass_utils import run_bass_kernel_spmd

    b, heads, h, w, d = 8, 8, 32, 32, 64
    q = np.ascontiguousarray(np.asarray(q, dtype=np.float32))
    k = np.ascontiguousarray(np.asarray(k, dtype=np.float32))
    v = np.ascontiguousarray(np.asarray(v, dtype=np.float32))
    assert q.shape == (b, heads, h, w, d), q.shape

    nc = _get_nc()
    in_maps = [
        {
            "q": q[c].reshape(heads, h * w, d),
            "k": k[c].reshape(heads, h * w, d),
            "v": v[c].reshape(heads, h * w, d),
        }
        for c in range(b)
    ]
    res = run_bass_kernel_spmd(nc, in_maps, core_ids=list(range(b)), trace=trace)
    out = np.stack(
        [res.results[c]["out"].reshape(h, w, heads * d) for c in range(b)]
    )
    return out, res


def kernel(q, k, v):
    out, _ = _run(q, k, v)
    return out

